# revision 11
# baseline (speedup 1.0000x reference)
"""ChannelSelfAttn Trainium2 kernel (bf16 redesign).

Reference computation (per sample b, x_b: [C=64, T=4000]):
    q = w1*x + b1, k = w2*x + b2 broadcast over F=16 feature maps
    e[i,j] = sum_{f,t} q[f,i,t]*k[f,j,t]
           = A*G[i,j] + B1*s_i + B2*s_j + C0*T
      where G = x_b @ x_b.T, s = rowsum(x_b),
            A = w1.w2, B1 = w1.b2, B2 = b1.w2, C0 = b1.b2
    e <- (e - min_j e)/(max_j e - min_j e + 1e-8)   # row terms cancel
    e <- softmax_j(e)
    out = gamma * (e @ x_b) + x_b

Only f = A*G + B2*s_j survives the normalize; everything reduces to a
64x64 gram matrix + row-softmax + a second small matmul per sample.

bf16 design (vs the f32r baseline):
  - x is converted to bf16 on the host; all DMA (in and out) is bf16,
    halving HBM traffic (4.1MB/core instead of 8.2MB).
  - all matmul operands are bf16 so every LDWEIGHTS gets fast-weight-load
    (~53ns vs ~170ns for fp32-mode loads, which dominated the baseline).
  - the attention weights are finalized as blk = (gamma/se)*gw^T + I, so
    one matmul computes gamma*softmax(e)@x + x directly and the out-path
    is a plain PSUM->SBUF cast.
  - row-sum s rides a ones-column in the xT layout (psum col 128 of the
    gram accumulates s for free).
  - small SBUF-only softmax ops go to GPSIMD; PSUM-touching copies split
    between DVE and ACT.

Sharding: data-parallel over batch. 32 samples / 8 cores = 4 samples/core,
processed as 2 pairs; each pair stacks 2 samples' channels into the 128
SBUF partitions.
"""

import numpy as np

import concourse.bacc as bacc
import concourse.bass as bass
import concourse.mybir as mybir
import concourse.tile as tile
from concourse.bass_utils import run_bass_kernel_spmd

FP32 = mybir.dt.float32
BF16 = mybir.dt.bfloat16
AF = mybir.ActivationFunctionType
ALU = mybir.AluOpType
AX = mybir.AxisListType

B, C, T = 32, 64, 4000
N_CORES = 8
SPC = B // N_CORES          # samples per core = 4
PAIRS = SPC // 2            # 2
TPAD = 4096                 # T padded to 32 chunks of 128
NCHUNK = TPAD // 128        # 32
NATT = TPAD // 512          # 8 attention N-chunks
CW = 129                    # xT chunk stride (128 data + 1 ones col)
XTW = NCHUNK * CW           # 4128


def build_program_bf16(A: float, B2: float, gamma: float, replicate: int = 1,
                       pt_cols: int = 1024, pa_cols: int = 1024,
                       gp_smalls: bool = True, in_pieces: int = 4,
                       out_pieces: int = 4, x_bufs: int = 3) -> bass.Bass:
    NGRP = NCHUNK * 128 // pt_cols          # transpose groups per pair
    TPG = pt_cols // 128                    # transposes per group
    MMPA = pa_cols // 512                   # matmuls per attn psum tile

    nc = bacc.Bacc(None)
    x_h = nc.declare_dram_parameter("x", [SPC * C, T], BF16, isOutput=False)
    id_h = nc.declare_dram_parameter("ident", [128, 128], BF16, isOutput=False)
    out_h = nc.declare_dram_parameter("out", [SPC * C, T], BF16, isOutput=True)

    def gp(default_eng):
        return nc.gpsimd if gp_smalls else default_eng

    with tile.TileContext(nc) as tc:
        with (
            tc.tile_pool(name="xio", bufs=x_bufs) as p_x,
            tc.tile_pool(name="xT", bufs=2) as p_xT,
            tc.tile_pool(name="outb", bufs=2) as p_out,
            tc.tile_pool(name="small", bufs=2) as p_small,
            tc.tile_pool(name="const", bufs=1) as p_const,
            tc.tile_pool(name="pt", bufs=2, space="PSUM") as p_pt,
            tc.tile_pool(name="pg", bufs=1, space="PSUM") as p_pg,
            tc.tile_pool(name="pa", bufs=2, space="PSUM") as p_pa,
            tc.tile_pool(name="ps", bufs=1, space="PSUM") as p_ps,
        ):
            identb = p_const.tile([128, 128], BF16)
            nc.sync.dma_start(identb[:], id_h[:, :])
            ones_b = p_const.tile([1, 128], BF16)
            nc.vector.memset(ones_b[:], 1.0)
            c_ones = p_const.tile([128, NCHUNK], BF16)
            nc.vector.memset(c_ones[:], 1.0)

            for p in [pp for _ in range(replicate) for pp in range(PAIRS)]:
                rows = slice(p * 128, (p + 1) * 128)

                # ---- load x pair [128, 4000] bf16, zero-pad t to 4096
                x_stack = p_x.tile([128, TPAD], BF16)
                ipc = T // in_pieces
                for i in range(in_pieces):
                    c0, c1 = i * ipc, (i + 1) * ipc
                    nc.sync.dma_start(x_stack[:, c0:c1], x_h[rows, c0:c1])
                gp(nc.vector).memset(x_stack[:, T:TPAD], 0.0)

                # ---- xT layout: per chunk [128 data cols][1 ones col]
                xT = p_xT.tile([128, XTW], BF16)
                oview = xT[:].rearrange("p (k c) -> p k c", c=CW)
                gp(nc.vector).tensor_copy(
                    oview[:, :, 128:129],
                    c_ones[:].rearrange("p (k o) -> p k o", o=1))

                # ---- transpose to xT [t, c] chunks via PE
                for q in range(NGRP):
                    pt = p_pt.tile([128, pt_cols], BF16, tag="pt")
                    for j in range(TPG):
                        k = TPG * q + j
                        nc.tensor.transpose(
                            pt[:, j * 128:(j + 1) * 128],
                            x_stack[:, k * 128:(k + 1) * 128],
                            identb[:, :],
                        )
                    dst = xT[:, q * TPG * CW:(q * TPG + TPG) * CW].rearrange(
                        "p (k c) -> p k c", c=CW)[:, :, 0:128]
                    src = pt[:].rearrange("p (k c) -> p k c", c=128)
                    if q % 2 == 0:
                        nc.vector.tensor_copy(dst, src)
                    else:
                        nc.scalar.copy(dst, src)

                # ---- gram accumulate; psum col 128 accumulates s = rowsum(x)
                pg = p_pg.tile([128, CW], FP32, tag="pg")
                for k in range(NCHUNK):
                    nc.tensor.matmul(
                        pg[:], lhsT=xT[:, k * CW:k * CW + 128],
                        rhs=xT[:, k * CW:k * CW + CW],
                        start=(k == 0), stop=(k == NCHUNK - 1),
                    )

                # ---- B2*s as a row via transpose + rank-1 broadcast matmul
                s_colr = p_small.tile([128, 1], BF16, tag="scolr")
                nc.scalar.mul(s_colr[:], pg[:, 128:129], B2)
                ps_row = p_ps.tile([1, 128], BF16, tag="psr")
                nc.tensor.transpose(ps_row[:], s_colr[:], identb[:, :])
                srow_b = p_small.tile([1, 128], BF16, tag="srowb")
                nc.vector.tensor_copy(srow_b[:], ps_row[:])
                psb = p_ps.tile([128, 128], FP32, tag="psb")
                nc.tensor.matmul(psb[:], lhsT=ones_b[:], rhs=srow_b[:],
                                 start=True, stop=True)
                sbs = p_small.tile([128, C], FP32, tag="sbs")
                nc.scalar.copy(sbs[0:64, :], psb[0:64, 0:64])
                nc.scalar.copy(sbs[64:128, :], psb[64:128, 64:128])

                # ---- f = A*G + B2*s_j (diag blocks only), stacked [128, 64]
                fs = p_small.tile([128, C], FP32, tag="fs")
                nc.vector.scalar_tensor_tensor(
                    fs[0:64, :], pg[0:64, 0:64], A, sbs[0:64, :],
                    op0=ALU.mult, op1=ALU.add,
                )
                nc.vector.scalar_tensor_tensor(
                    fs[64:128, :], pg[64:128, 64:128], A, sbs[64:128, :],
                    op0=ALU.mult, op1=ALU.add,
                )

                # ---- row minmax-normalize + exp (+ rowsum for softmax denom)
                mx = p_small.tile([128, 1], FP32, tag="mx")
                nc.vector.reduce_max(mx[:], fs[:], axis=AX.X)
                mn = p_small.tile([128, 1], FP32, tag="mn")
                nc.vector.tensor_reduce(mn[:], fs[:], axis=AX.X, op=ALU.min)
                dd = p_small.tile([128, 1], FP32, tag="dd")
                nc.vector.scalar_tensor_tensor(
                    dd[:], mx[:], 1e-8, mn[:], op0=ALU.add, op1=ALU.subtract,
                )
                rr = p_small.tile([128, 1], FP32, tag="rr")
                nc.vector.reciprocal(rr[:], dd[:])
                nb = p_small.tile([128, 1], FP32, tag="nb")
                nc.vector.scalar_tensor_tensor(
                    nb[:], mn[:], -1.0, rr[:], op0=ALU.mult, op1=ALU.mult,
                )
                gw = p_small.tile([128, 128], BF16, tag="gw")
                gp(nc.vector).memset(gw[:], 0.0)
                se = p_small.tile([128, 1], FP32, tag="se")
                nc.scalar.activation(
                    gw[0:64, 0:64], fs[0:64, :], AF.Exp,
                    bias=nb[0:64], scale=rr[0:64], accum_out=se[0:64],
                )
                nc.scalar.activation(
                    gw[64:128, 64:128], fs[64:128, :], AF.Exp,
                    bias=nb[64:128], scale=rr[64:128], accum_out=se[64:128],
                )
                rs = p_small.tile([128, 1], FP32, tag="rs")
                nc.vector.reciprocal(rs[:], se[:])
                wsc = p_small.tile([128, 1], FP32, tag="wsc")
                nc.vector.tensor_scalar_mul(wsc[:], rs[:], gamma)

                # ---- scale rows by gamma/se, transpose, add I:
                #      blk = (gamma/se)*gw^T + I  so  blk^T@x = gamma*e@x + x
                nc.scalar.mul(gw[0:64, 0:64], gw[0:64, 0:64], wsc[0:64])
                nc.scalar.mul(gw[64:128, 64:128], gw[64:128, 64:128],
                              wsc[64:128])
                pb = p_ps.tile([128, 128], BF16, tag="pb")
                nc.tensor.transpose(pb[:], gw[:], identb[:, :])
                blk = p_small.tile([128, 128], BF16, tag="blk")
                nc.vector.tensor_add(blk[:], pb[:], identb[:])

                # ---- out = blk^T @ x  (= gamma*attn + x), cast to bf16
                ob = p_out.tile([128, TPAD], BF16)
                for n in range(NATT // MMPA):
                    pa = p_pa.tile([128, pa_cols], FP32, tag="pa")
                    for j in range(MMPA):
                        c0 = (n * MMPA + j) * 512
                        nc.tensor.matmul(pa[:, j * 512:(j + 1) * 512],
                                         lhsT=blk[:],
                                         rhs=x_stack[:, c0:c0 + 512],
                                         start=True, stop=True)
                    obc = ob[:, n * pa_cols:(n + 1) * pa_cols]
                    if n % 2 == 0:
                        nc.vector.tensor_copy(obc, pa[:])
                    else:
                        nc.scalar.copy(obc, pa[:])

                opc = T // out_pieces
                for i in range(out_pieces):
                    c0, c1 = i * opc, (i + 1) * opc
                    nc.sync.dma_start(out_h[rows, c0:c1], ob[:, c0:c1])

    nc.finalize()
    return nc


# Final kernel configuration (selected by on-hardware benchmarking)
BUILD = build_program_bf16
BUILD_KWARGS = {"pa_cols": 512}

BF16_NP = mybir.dt.np(BF16)


def make_in_maps(x):
    """x: [B,1,C,T] float anything -> per-core input dicts (bf16)."""
    x = np.asarray(x, dtype=np.float32)
    xs = x[:, 0].reshape(N_CORES, SPC * C, T).astype(BF16_NP)
    eye = np.eye(128, dtype=np.float32).astype(BF16_NP)
    return [{"x": np.ascontiguousarray(xs[r]), "ident": eye}
            for r in range(N_CORES)]


def _run(x, w1, b1, w2, b2, gamma, **run_kwargs):
    x = np.ascontiguousarray(np.asarray(x, dtype=np.float32))
    w1 = np.asarray(w1, dtype=np.float32)
    b1 = np.asarray(b1, dtype=np.float32)
    w2 = np.asarray(w2, dtype=np.float32)
    b2 = np.asarray(b2, dtype=np.float32)
    gamma = np.asarray(gamma, dtype=np.float32)
    assert x.shape == (B, 1, C, T), x.shape

    A = float(w1 @ w2)
    B2c = float(b1 @ w2)
    gam = float(gamma.reshape(-1)[0])

    nc = BUILD(A, B2c, gam, **BUILD_KWARGS)
    in_maps = make_in_maps(x)
    res = run_bass_kernel_spmd(nc, in_maps, list(range(N_CORES)), **run_kwargs)
    out = np.stack([np.asarray(res.results[r]["out"], dtype=np.float32)
                    for r in range(N_CORES)])
    out = out.reshape(B, C, T)[:, None].astype(np.float32)
    return out, res


def kernel(x, w1, b1, w2, b2, gamma):
    out, _ = _run(x, w1, b1, w2, b2, gamma)
    return out


def make_timed_runner(nc, in_maps):
    """Build a jitted 8-core runner (no donation) for repeat timing.

    Mirrors bass2jax.run_bass_via_pjrt's multi-core path but keeps the jitted
    function so the NEFF can be executed repeatedly with device-resident args.
    """
    import jax
    import numpy as _np
    from jax.sharding import Mesh, PartitionSpec
    from jax.experimental.shard_map import shard_map

    import concourse.mybir as _mybir
    from concourse import bass2jax
    from concourse.bass2jax import _bass_exec_p, install_neuronx_cc_hook

    install_neuronx_cc_hook()
    n_cores = len(in_maps)
    partition_name = nc.partition_id_tensor.name if nc.partition_id_tensor else None

    in_names, out_names, out_avals, zero_outs = [], [], [], []
    for alloc in nc.m.functions[0].allocations:
        if not isinstance(alloc, _mybir.MemoryLocationSet):
            continue
        name = alloc.memorylocations[0].name
        if alloc.kind == "ExternalInput":
            if name != partition_name:
                in_names.append(name)
        elif alloc.kind == "ExternalOutput":
            out_names.append(name)
            shape = tuple(alloc.tensor_shape)
            dtype = _mybir.dt.np(alloc.dtype)
            out_avals.append(jax.core.ShapedArray(shape, dtype))
            zero_outs.append(_np.zeros(shape, dtype))
    n_params = len(in_names)
    in_names = in_names + out_names
    if partition_name is not None:
        in_names.append(partition_name)

    def _exec_once(*args):
        operands = list(args)
        if partition_name is not None:
            operands.append(bass2jax.partition_id_tensor())
        outs = _bass_exec_p.bind(
            *operands,
            out_avals=tuple(out_avals),
            in_names=tuple(in_names),
            out_names=tuple(out_names),
            lowering_input_output_aliases=(),
            sim_require_finite=True,
            sim_require_nnan=True,
            nc=nc,
        )
        return tuple(outs)

    assert len(out_names) == 1

    devices = jax.devices()[:n_cores]
    mesh = Mesh(_np.asarray(devices), ("core",))
    in_specs = (PartitionSpec("core"),) * (n_params + len(out_names))
    out_specs = (PartitionSpec("core"),) * len(out_names)
    fn = jax.jit(
        shard_map(_exec_once, mesh=mesh, in_specs=in_specs, out_specs=out_specs,
                  check_rep=False),
        keep_unused=True,
    )
    concat_in = [
        _np.concatenate([_np.asarray(in_maps[c][nm]) for c in range(n_cores)], axis=0)
        for nm in in_names[:n_params]
    ]
    concat_zeros = [
        _np.zeros((n_cores * z.shape[0], *z.shape[1:]), z.dtype) for z in zero_outs
    ]
    shard = jax.sharding.NamedSharding(mesh, PartitionSpec("core"))
    args = [jax.device_put(a, shard) for a in concat_in + concat_zeros]

    def run():
        o = fn(*args)[0]
        return jax.block_until_ready(o)

    run.fn = fn
    run.args = args
    return run, out_names, out_avals


def timed_run(x, w1, b1, w2, b2, gamma, r1=2, r2=10, reps=15,
              build=None):
    """Measure per-kernel device time via the slope between two NEFFs that
    run the whole kernel body `r1` and `r2` times internally (the constant
    axon RPC overhead cancels in the difference)."""
    import time as _time

    x = np.ascontiguousarray(np.asarray(x, dtype=np.float32))
    A = float(np.asarray(w1, np.float32) @ np.asarray(w2, np.float32))
    B2c = float(np.asarray(b1, np.float32) @ np.asarray(w2, np.float32))
    gam = float(np.asarray(gamma, np.float32).reshape(-1)[0])
    in_maps = make_in_maps(x)

    t_best = {}
    out_arr = None
    out_avals = None
    if build is None:
        def build(A_, B2_, g_, replicate=1):
            return BUILD(A_, B2_, g_, replicate=replicate, **BUILD_KWARGS)
    for rep in (r1, r2):
        nc = build(A, B2c, gam, replicate=rep)
        run, out_names, out_avals = make_timed_runner(nc, in_maps)
        out_arr = run()  # compile + warmup
        run()
        best = None
        for _ in range(reps):
            t0 = _time.perf_counter_ns()
            run()
            dt = _time.perf_counter_ns() - t0
            best = dt if best is None else min(best, dt)
        t_best[rep] = best

    per_exec_ns = (t_best[r2] - t_best[r1]) / (r2 - r1)
    out = np.asarray(out_arr, dtype=np.float32)
    out = out.reshape(N_CORES, *out_avals[0].shape).reshape(B, C, T)[:, None]
    return out.astype(np.float32), per_exec_ns
